# revision 18
# baseline (speedup 1.0000x reference)
"""ColBERT late-interaction kernel for 8 Trainium2 NeuronCores.

Math (per reference):
  x = h @ W + b                      (projection, H=768 -> D=128)
  v = x / ||x||_2(seq axis)          (normalize over the SEQUENCE axis)
  sim[q,p,n,l] = <q_v[q,n], p_v[p,l]>
  scores[q,p] = sum_n max_{l valid} sim[q,p,n,l]
  out = concat(pos_scores, neg_scores, axis=1)   # [96, 192]

Sharding: passage-parallel. Every core projects ALL queries (cheap) and a
1/8 shard of pos+neg passages (12+12 batches), computes the full-query x
local-passage score block [96, 24], and the host stitches columns.

Device layout notes:
  - Hidden tensors + W ship in bf16 (halves HBM traffic; well within the
    2e-2 tolerance). All matmuls run bf16 at 1 col/cycle, fp32 PSUM accum.
  - Sequence-axis sum-of-squares is computed via TensorE: transpose x
    slices (tokens onto partitions), square them on Scalar while draining
    to SBUF, then a 0/1 indicator matmul sums over the token partitions,
    accumulating [batches, 128] in PSUM. One transpose-back + sqrt +
    reciprocal yields the per-(d,batch) inverse norms. This replaces the
    per-batch Square+accumulator-read chain that serialized Scalar.
  - All pointwise scaling (query normalize, passage normalize + mask) runs
    on GpSimd, keeping Vector free for the MaxSim reduce, which is the
    critical engine (1 elem/cycle/lane from PSUM).
  - Passages are sorted by valid-token count, valid tokens permuted to the
    front, so the MaxSim reduce reads only the live prefix of each segment.
  - The final sum-over-n is a ones-block matmul that also performs the
    cross-partition (query-token) reduction.
"""

import numpy as np

B, NQ, LP, H, D = 96, 35, 180, 768, 128
NCORES = 8
PB = B // NCORES          # 12 passage batches per core per side
LOCAL_P = 2 * PB          # 24 local passage batches (pos then neg)
QCOLS = B * NQ            # 3360 query columns
PCOLS = LOCAL_P * LP      # 4320 passage columns
KCH = H // 128            # 6 contraction chunks
QCHUNK = 420              # 12 query batches per projection chunk
NQCH = QCOLS // QCHUNK    # 8
PCHUNK = 360              # 2 passage batches per projection chunk
NPCH = PCOLS // PCHUNK    # 12
NGROUPS = (QCOLS + 127) // 128       # 27 interaction row-groups
BPT = 6                   # passage batches per sim tile (3 psum banks)
NSIMTILES = LOCAL_P // BPT           # 4
PW = BPT * LP             # passage cols per sim tile (1080)
NQSL = 4                  # 128-col transpose slices per q chunk (420)
NPSL = 9                  # 128-col transpose slices per p tile (1080)


def _build(tile_lens):
    import concourse.bacc as bacc
    from concourse import mybir
    from concourse.tile import TileContext

    f32 = mybir.dt.float32
    bf16 = mybir.dt.bfloat16

    nc = bacc.Bacc(target_bir_lowering=False)

    QH = nc.dram_tensor("qh", [NQCH, 128, KCH * QCHUNK], bf16,
                        kind="ExternalInput")
    PH = nc.dram_tensor("ph", [NPCH, 128, KCH * PCHUNK], bf16,
                        kind="ExternalInput")
    WT = nc.dram_tensor("w", [128, KCH * D], bf16, kind="ExternalInput")
    BT = nc.dram_tensor("bias", [D, 1], f32, kind="ExternalInput")
    ONES = nc.dram_tensor("ones", [128, NGROUPS * B], bf16, kind="ExternalInput")
    MASK = nc.dram_tensor("mask", [128, PCOLS], bf16, kind="ExternalInput")
    OUT = nc.dram_tensor("scores", [B, LOCAL_P], f32, kind="ExternalOutput")
    Ident = mybir.ActivationFunctionType.Identity
    Square = mybir.ActivationFunctionType.Square
    MUL = mybir.AluOpType.mult
    ADD = mybir.AluOpType.add
    MAXOP = mybir.AluOpType.max

    with TileContext(nc) as tc:
        with (
            tc.tile_pool(name="consts", bufs=1) as consts,
            tc.tile_pool(name="hidp", bufs=4) as hidp,
            tc.tile_pool(name="xbuf", bufs=1) as xbuf,
            tc.tile_pool(name="stats", bufs=1) as stats,
            tc.tile_pool(name="rnp", bufs=2) as rnp,
            tc.tile_pool(name="sqp", bufs=2) as sqp,
            tc.tile_pool(name="mxp", bufs=NGROUPS) as mxp,
            tc.tile_pool(name="ps_w", bufs=2, space="PSUM") as ps_w,
            tc.tile_pool(name="offp", bufs=2) as offp,
            tc.tile_pool(name="ps_sim", bufs=2, space="PSUM") as ps_sim,
        ):
            w_t = consts.tile([128, KCH, D], bf16, tag="w")
            nc.sync.dma_start(
                out=w_t[:], in_=WT[:].rearrange("p (k d) -> p k d", d=D)
            )
            b_t = consts.tile([D, 1], f32, tag="b")
            nc.sync.dma_start(out=b_t[:], in_=BT[:])
            mask_t = consts.tile([128, PCOLS], bf16, tag="mask")
            nc.sync.dma_start(out=mask_t[:, :PW], in_=MASK[:, :PW])

            xp = xbuf.tile([128, PCOLS], bf16, tag="xp")
            xq = xbuf.tile([128, QCOLS], bf16, tag="xq")
            xqn = xbuf.tile([128, QCOLS], bf16, tag="xqn")
            xpn = xbuf.tile([128, PCOLS], bf16, tag="xpn")

            def proj_chunk(src, cidx, lo, ncols, xdst):
                """Project ncols starting at lo; ACT adds bias while
                draining PSUM to bf16 SBUF."""
                hid = hidp.tile([128, KCH, QCHUNK], bf16, tag="hid")
                hid_v = hid[:, :, :ncols]
                nc.sync.dma_start(
                    out=hid_v,
                    in_=src[cidx].rearrange("p (k n) -> p k n", k=KCH),
                )
                ps = ps_w.tile([128, QCHUNK], f32, tag="w")
                ps_v = ps[:, :ncols]
                for k in range(KCH):
                    nc.tensor.matmul(
                        ps_v, w_t[:, k, :], hid_v[:, k, :],
                        start=(k == 0), stop=(k == KCH - 1),
                    )
                nc.scalar.activation(
                    xdst[:, lo:lo + ncols], ps_v, Ident, bias=b_t[:, 0:1]
                )

            def norm_chain(xsrc, lo, ncols, nb, seg):
                """Inverse seq-norms for the nb batches in xsrc[:, lo:lo+ncols]:
                Square-drain the bf16 x (Scalar), strided sum per batch
                (Vector, cheap), then sqrt + reciprocal."""
                sq = sqp.tile([128, PW], bf16, tag="sq")
                nc.scalar.activation(sq[:, :ncols], xsrc[:, lo:lo + ncols],
                                     Square)
                ssv = rnp.tile([128, 12], f32, tag="ssv")
                nc.vector.tensor_reduce(
                    out=ssv[:, :nb],
                    in_=sq[:, :ncols].rearrange("p (b n) -> p b n", n=seg),
                    axis=mybir.AxisListType.X, op=ADD,
                )
                rt = rnp.tile([128, 12], f32, tag="rt")
                nc.scalar.sqrt(rt[:, :nb], ssv[:, :nb])
                y0 = rnp.tile([128, 12], f32, tag="y0")
                nc.vector.reciprocal(y0[:, :nb], rt[:, :nb])
                return y0

            def q_chunk(c):
                proj_chunk(QH, c, c * QCHUNK, QCHUNK, xq)
                rq = norm_chain(xq, c * QCHUNK, QCHUNK, 12, NQ)
                lo = c * QCHUNK
                nc.gpsimd.tensor_tensor(
                    out=xqn[:, lo:lo + QCHUNK].rearrange(
                        "p (b n) -> p b n", n=NQ),
                    in0=xq[:, lo:lo + QCHUNK].rearrange(
                        "p (b n) -> p b n", n=NQ),
                    in1=rq[:, :12].to_broadcast([128, 12, NQ]),
                    op=MUL,
                )

            def p_norm(t):
                """Mask+scale tile t's 6 batches on GpSimd, in 2-batch
                pieces so the first sim matmul can fire early."""
                rp = norm_chain(xp, t * PW, PW, BPT, LP)
                for cc in range(3):
                    lo = t * PW + cc * PCHUNK
                    nc.gpsimd.tensor_tensor(
                        out=xpn[:, lo:lo + PCHUNK], in0=xp[:, lo:lo + PCHUNK],
                        in1=mask_t[:, lo:lo + PCHUNK], op=MUL,
                    )
                    nc.gpsimd.tensor_tensor(
                        out=xpn[:, lo:lo + PCHUNK].rearrange(
                            "p (b n) -> p b n", n=LP),
                        in0=xpn[:, lo:lo + PCHUNK].rearrange(
                            "p (b n) -> p b n", n=LP),
                        in1=rp[:, 2 * cc:2 * cc + 2].to_broadcast(
                            [128, 2, LP]),
                        op=MUL,
                    )

            def p_tile(t):
                for cc in range(NPCH // NSIMTILES):
                    c = t * (NPCH // NSIMTILES) + cc
                    proj_chunk(PH, c, c * PCHUNK, PCHUNK, xp)
                p_norm(t)

            # ---- wavefront ----
            mx_tiles = {}
            next_t = [0] * NGROUPS
            scsum = stats.tile([B, LOCAL_P], f32, tag="scsum")
            nc.vector.memset(scsum[:], 0.0)

            def emit_pair(g, t):
                rows = min(128, QCOLS - g * 128)
                lhs = xqn[:, g * 128:g * 128 + rows]
                if g not in mx_tiles:
                    mx_tiles[g] = mxp.tile(
                        [128, LOCAL_P], bf16, tag="mx", name=f"mx{g}"
                    )
                mx = mx_tiles[g]
                vlen = tile_lens[t]
                sim = ps_sim.tile([128, 3 * 512], f32, tag="sim")
                sim_b = sim[:rows].rearrange("p (k b) -> p k b", b=512)
                for j in range(3):
                    pc0 = (t * BPT + 2 * j) * LP
                    nc.tensor.matmul(
                        sim_b[:, j, :PCHUNK], lhs, xpn[:, pc0:pc0 + PCHUNK],
                        start=True, stop=True,
                    )
                if False:
                    # offload: Scalar drains sim to SBUF bf16, GpSimd runs a
                    # 4-level pairwise max tree, Vector finishes cheaply
                    scr = offp.tile([128, 3, PCHUNK], bf16, tag="scr")
                    nc.scalar.activation(
                        scr[:rows], sim_b[:rows, :, :PCHUNK], Ident
                    )
                    srcv = scr[:rows].rearrange("p k (s l) -> p (k s) l", l=LP)
                    tr = offp.tile([128, BPT, 96], bf16, tag="tr")
                    tr2 = offp.tile([128, BPT, 48], bf16, tag="tr2")
                    v = vlen
                    c1 = (v + 1) // 2
                    nc.gpsimd.tensor_tensor(
                        out=tr[:rows, :, :c1], in0=srcv[:, :, :c1],
                        in1=srcv[:, :, v - c1:v], op=MAXOP,
                    )
                    c2 = (c1 + 1) // 2
                    nc.gpsimd.tensor_tensor(
                        out=tr2[:rows, :, :c2], in0=tr[:rows, :, :c2],
                        in1=tr[:rows, :, c1 - c2:c1], op=MAXOP,
                    )
                    c3 = (c2 + 1) // 2
                    nc.gpsimd.tensor_tensor(
                        out=tr[:rows, :, :c3], in0=tr2[:rows, :, :c3],
                        in1=tr2[:rows, :, c2 - c3:c2], op=MAXOP,
                    )
                    c4 = (c3 + 1) // 2
                    nc.gpsimd.tensor_tensor(
                        out=tr2[:rows, :, :c4], in0=tr[:rows, :, :c4],
                        in1=tr[:rows, :, c3 - c4:c3], op=MAXOP,
                    )
                    nc.vector.reduce_max(
                        mx[:rows, t * BPT:(t + 1) * BPT],
                        tr2[:rows, :, :c4], axis=mybir.AxisListType.X,
                    )
                else:
                    sim_seg = sim_b[:, :, :PCHUNK].rearrange(
                        "p k (s l) -> p k s l", l=LP
                    )[:, :, :, :vlen]
                    nc.vector.reduce_max(
                        mx[:rows, t * BPT:(t + 1) * BPT], sim_seg,
                        axis=mybir.AxisListType.X,
                    )
                if t == NSIMTILES - 1:
                    nsum = ps_w.tile([B, LOCAL_P], f32, tag="w")
                    nc.tensor.matmul(
                        nsum[:], ones_t[:rows, g, :], mx[:rows, :],
                        start=True, stop=True,
                    )
                    nsb = rnp.tile([B, LOCAL_P], f32, tag="nsb")
                    nc.scalar.copy(nsb[:], nsum[:])
                    nc.gpsimd.tensor_tensor(
                        out=scsum[:], in0=scsum[:], in1=nsb[:], op=ADD,
                    )

            def flush(q_cols_done, p_tiles_done):
                for g in range(NGROUPS):
                    rows = min(128, QCOLS - g * 128)
                    if g * 128 + rows > q_cols_done:
                        break
                    while next_t[g] < p_tiles_done:
                        emit_pair(g, next_t[g])
                        next_t[g] += 1

            # ---- phase A: p tile 0 and q chunk 0 first, then stream ----
            for cc in range(NPCH // NSIMTILES):
                proj_chunk(PH, cc, cc * PCHUNK, PCHUNK, xp)
            q_chunk(0)
            p_norm(0)
            flush(QCHUNK, 1)            # g0-g2 x t0

            q_chunk(1)
            flush(2 * QCHUNK, 1)        # g3-g5 x t0

            nc.sync.dma_start(
                out=mask_t[:, PW:2 * PW], in_=MASK[:, PW:2 * PW]
            )
            p_tile(1)
            q_chunk(2)
            flush(3 * QCHUNK, 2)

            q_chunk(3)
            flush(4 * QCHUNK, 2)

            nc.sync.dma_start(
                out=mask_t[:, 2 * PW:3 * PW], in_=MASK[:, 2 * PW:3 * PW]
            )
            ones_t = consts.tile([128, NGROUPS, B], bf16, tag="ones")
            nc.sync.dma_start(
                out=ones_t[:],
                in_=ONES[:].rearrange("p (g q) -> p g q", q=B),
            )
            p_tile(2)
            q_chunk(4)
            flush(5 * QCHUNK, 3)

            q_chunk(5)
            flush(6 * QCHUNK, 3)

            nc.sync.dma_start(
                out=mask_t[:, 3 * PW:4 * PW], in_=MASK[:, 3 * PW:4 * PW]
            )
            p_tile(3)
            q_chunk(6)
            flush(7 * QCHUNK, 4)

            q_chunk(7)
            flush(QCOLS, NSIMTILES)

            nc.sync.dma_start(out=OUT[:], in_=scsum[:])

    nc.compile()
    return nc


def _prepare(q_hidden, pos_hidden, neg_hidden, W, b, pos_mask, neg_mask):
    """Shard + transpose inputs on host. Returns (in_maps, orders, tile_lens)."""
    import ml_dtypes

    bf16 = ml_dtypes.bfloat16

    def chunk_major(hT, nch, chunk):
        v = hT.reshape(KCH, 128, nch, chunk)
        return np.ascontiguousarray(
            v.transpose(2, 1, 0, 3).reshape(nch, 128, KCH * chunk)
        ).astype(bf16)

    qhT = q_hidden.transpose(2, 0, 1).reshape(H, QCOLS).astype(np.float32)
    qh_c = chunk_major(qhT, NQCH, QCHUNK)
    Wc = np.ascontiguousarray(
        np.asarray(W, dtype=np.float32).reshape(KCH, 128, D)
        .transpose(1, 0, 2).reshape(128, KCH * D)
    ).astype(bf16)
    bc = np.ascontiguousarray(b, dtype=np.float32).reshape(D, 1)

    ones = np.zeros((128, NGROUPS * B), dtype=np.float32)
    for g in range(NGROUPS):
        rows = min(128, QCOLS - g * 128)
        for r in range(rows):
            qb = (g * 128 + r) // NQ
            ones[r, g * B + qb] = 1.0
    ones = ones.astype(bf16)


    per_core = []
    all_V = np.zeros((NCORES, LOCAL_P), dtype=np.int64)
    for i in range(NCORES):
        sl = slice(i * PB, (i + 1) * PB)
        h_loc = np.concatenate([pos_hidden[sl], neg_hidden[sl]], axis=0)
        m_loc = np.concatenate([pos_mask[sl], neg_mask[sl]], axis=0)
        V = m_loc.sum(axis=1).astype(np.int64)            # [24]
        order = np.argsort(-V, kind="stable")             # big batches first
        phT = np.empty((H, PCOLS), dtype=np.float32)
        mrow = np.empty(PCOLS, dtype=np.float32)
        for j, lb in enumerate(order):
            perm = np.concatenate(
                [np.flatnonzero(m_loc[lb]), np.flatnonzero(~m_loc[lb])]
            )
            phT[:, j * LP:(j + 1) * LP] = h_loc[lb][perm].T
            mrow[j * LP:(j + 1) * LP] = m_loc[lb][perm]
        all_V[i] = V[order]
        mask_full = np.ascontiguousarray(
            np.broadcast_to(mrow[None, :], (128, PCOLS))
        ).astype(bf16)
        per_core.append((phT, order, mask_full))

    tile_lens = []
    for t in range(NSIMTILES):
        tile_lens.append(int(all_V[:, t * BPT].max()))

    in_maps = []
    orders = []
    for i in range(NCORES):
        phT, order, mask_full = per_core[i]
        in_maps.append({
            "qh": qh_c, "ph": chunk_major(phT, NPCH, PCHUNK),
            "w": Wc, "bias": bc, "ones": ones, "mask": mask_full,
        })
        orders.append(order)
    return in_maps, orders, tile_lens


def _assemble(results, orders):
    out = np.zeros((B, 2 * B), dtype=np.float32)
    for i in range(NCORES):
        sc = results[i]["scores"]                          # [96, 24]
        for j, lb in enumerate(orders[i]):
            if lb < PB:
                out[:, i * PB + lb] = sc[:, j]
            else:
                out[:, B + i * PB + (lb - PB)] = sc[:, j]
    return out


def _run(inputs, trace=False):
    from concourse.bass_utils import run_bass_kernel_spmd

    in_maps, orders, tile_lens = _prepare(**inputs)
    nc = _build(tuple(tile_lens))
    kw = {}
    if trace:
        import os
        import shutil

        tmpdir = "/tmp/trn_trace"
        shutil.rmtree(tmpdir, ignore_errors=True)
        os.makedirs(tmpdir, exist_ok=True)
        kw["tmpdir"] = tmpdir
    res = run_bass_kernel_spmd(nc, in_maps, list(range(NCORES)), trace=trace, **kw)
    return _assemble(res.results, orders), res


def kernel(**inputs) -> np.ndarray:
    out, _ = _run(inputs, trace=False)
    return out


def kernel_profiled(**inputs):
    out, res = _run(inputs, trace=True)
    return out, res
